# revision 7
# baseline (speedup 1.0000x reference)
"""Trainium2 Bass kernel for nn_Copy_56470230008202 (sparse_attention).

Strategy (8 NeuronCores, SPMD, one launch, one AllGather):
  The reference's `mixh.reshape(1,-1,H)` / `q2 = qh.transpose(1,0,2,3).reshape(-1,1,H)`
  views scramble rows so that output row l' = n*128 + pg (head n, position
  group pg) draws features from positions t = pg*16 + j of head n only.
  Hence: core i owns heads {2i, 2i+1} == output rows [i*256, (i+1)*256).

  - conv0 (CIN->H, k=3): channel-sharded; core i computes x0 channels
    [128i, 128i+128) over all L from the (replicated) input o.
  - AllGather x0 (4 MB total) -> every core holds full x0.
  - conv1 (H->H, k=3): core i computes only its 128 q-channels (2 heads).
  - attention per head (scoresT layout [s', t], softmax over partitions via
    an appended ones-column in kv for the denominator; no max subtraction --
    scores are in [-6, 6]), mix accumulated over s' tiles.
  - scramble rewrite into cat layout via strided DVE writes.
  - out-proj + SELU -> aoT [c_out, l'], then V/C logits vs full VC^T,
    streamed; psum evacuated as bf16 and written to a bf16 output, cast to
    fp32 + bias on host.
  All matmuls bf16 inputs / fp32 PSUM accumulation. Weight-norm, selu(f),
  transposes, sharding and the final bias add run on host.
"""

import os
import sys

for _p in ("/opt/trn_rl_repo", "/root/.axon_site/_ro/trn_rl_repo"):
    if os.path.isdir(_p) and _p not in sys.path:
        sys.path.append(_p)

import numpy as np
import ml_dtypes

import concourse.bass as bass
import concourse.mybir as mybir
from concourse import bacc
from concourse.tile import TileContext
from concourse.bass_utils import run_bass_kernel_spmd

F32 = mybir.dt.float32
BF16 = mybir.dt.bfloat16
ALU = mybir.AluOpType
ACTF = mybir.ActivationFunctionType

H, NH, HD = 1024, 16, 64
CIN, VOCAB, LIMIT, L, S = 1280, 32000, 512, 2048, 2048
VC = VOCAB + LIMIT              # 32512 = 64 * 508
NVB, VBW = 64, 508              # vocab blocks
NCORES = 8
LAM, ALPHA = 1.0507009873554805, 1.6732632423543772

# "gather": conv0 channel-sharded + AllGather (fast).
# "replicate": every core computes the full conv0 (no collective; fallback).
MODE = os.environ.get("NN_COPY_MODE", "gather")


def _selu_from_psum(nc, tmp, zeros, psum_ap, bias_ap, out_ap, P, N, idx, pbase=0):
    """out = selu(z) given psum = LAM*z (lambda folded into weights+bias).
    selu(z) = max(y,0) + LAM*ALPHA*(exp(min(y,0)/LAM) - 1),  y = LAM*z + b'.
    pbase: base partition of bias_ap -- SBUF operands of one instruction
    must share their base partition (walrus NCC_IBIR297).
    """
    m = tmp.tile([P, N], F32, name=f"selu_m{idx}", tag=f"selu_m{P}x{N}")
    r = tmp.tile([P, N], F32, name=f"selu_r{idx}", tag=f"selu_r{P}x{N}")
    e = tmp.tile([P, N], F32, name=f"selu_e{idx}", tag=f"selu_e{P}x{N}")
    t = tmp.tile([P, N], F32, name=f"selu_t{idx}", tag=f"selu_t{P}x{N}")
    z = zeros[pbase:pbase + P, :N]
    nc.vector.scalar_tensor_tensor(m, psum_ap, bias_ap, z, op0=ALU.add, op1=ALU.min)
    nc.vector.scalar_tensor_tensor(r, psum_ap, bias_ap, z, op0=ALU.add, op1=ALU.max)
    nc.scalar.activation(e, m, ACTF.Exp, scale=1.0 / LAM)
    nc.vector.tensor_scalar(t, e, LAM * ALPHA, -LAM * ALPHA, op0=ALU.mult, op1=ALU.add)
    nc.vector.tensor_tensor(out_ap, t, r, op=ALU.add)


def build_program(mode=MODE):
    nc = bacc.Bacc("TRN2", target_bir_lowering=False, debug=False,
                   num_devices=NCORES)
    oT = nc.declare_dram_parameter("oT", [CIN, L + 2], BF16, isOutput=False)
    if mode == "gather":
        w0T = nc.declare_dram_parameter("w0T", [3 * CIN, 128], BF16, isOutput=False)
        q0b = nc.declare_dram_parameter("q0b", [128, 1], F32, isOutput=False)
    else:
        w0T = nc.declare_dram_parameter("w0T", [3 * CIN, H], BF16, isOutput=False)
        q0b = nc.declare_dram_parameter("q0b", [128, 8], F32, isOutput=False)
    w1T = nc.declare_dram_parameter("w1T", [3 * H, 128], BF16, isOutput=False)
    q1b = nc.declare_dram_parameter("q1b", [128, 1], F32, isOutput=False)
    kvT = nc.declare_dram_parameter("kvT", [128, S], BF16, isOutput=False)
    kvag = nc.declare_dram_parameter("kvag", [S, 130], BF16, isOutput=False)
    woT = nc.declare_dram_parameter("woT", [2 * H, H], BF16, isOutput=False)
    outb = nc.declare_dram_parameter("outb", [128, 8], F32, isOutput=False)
    vct = nc.declare_dram_parameter("vct", [H, VC], BF16, isOutput=False)
    out = nc.declare_dram_parameter("out", [256, VC], BF16, isOutput=True)

    with TileContext(nc) as tc:
        _emit(tc, mode, oT, w0T, q0b, w1T, q1b, kvT, kvag, woT, outb, vct, out)
    if not nc.is_finalized():
        nc.finalize()
    return nc


def _emit(tc, mode, oT, w0T, q0b, w1T, q1b, kvT, kvag, woT, outb, vct, out):
    nc = tc.nc

    with tc.tile_pool(name="const", bufs=1) as constp, \
         tc.tile_pool(name="persist", bufs=1) as pers, \
         tc.tile_pool(name="dram", bufs=1, space="DRAM") as dram:
        zeros = constp.tile([128, 512], F32)
        nc.vector.memset(zeros, 0.0)
        ones = constp.tile([1, 64], F32)
        nc.vector.memset(ones, 1.0)
        q0b_sb = constp.tile(list(q0b.shape), F32)
        nc.sync.dma_start(out=q0b_sb, in_=q0b[:, :])
        q1b_sb = constp.tile([128, 1], F32)
        nc.sync.dma_start(out=q1b_sb, in_=q1b[:, :])
        outb_sb = constp.tile([128, 8], F32)
        nc.sync.dma_start(out=outb_sb, in_=outb[:, :])

        # persistent activations
        qh = [pers.tile([64, L], BF16, name=f"qh{hh}") for hh in range(2)]
        catm = [[pers.tile([128, 128], BF16, name=f"catm{hh}_{kk}")
                 for kk in range(8)] for hh in range(2)]
        catq = [[pers.tile([128, 128], BF16, name=f"catq{hh}_{kk}")
                 for kk in range(8)] for hh in range(2)]
        aoT = [[pers.tile([128, 128], BF16, name=f"aoT{hh}_{kk}")
                for kk in range(8)] for hh in range(2)]
        kvT_sb = [pers.tile([64, S], BF16, name=f"kvT{hh}") for hh in range(2)]
        for hh in range(2):
            nc.sync.dma_start(out=kvT_sb[hh], in_=kvT[hh * 64:(hh + 1) * 64, :])
        kvag_sb = [pers.tile([128, 130], BF16, name=f"kvag{st}") for st in range(16)]
        for st in range(16):
            nc.sync.dma_start(out=kvag_sb[st], in_=kvag[st * 128:(st + 1) * 128, :])

        # ---------------- conv0 ----------------
        # x0 channels for this core (gather) or all channels (replicate)
        n_m0 = 1 if mode == "gather" else 8
        with tc.tile_pool(name="c0", bufs=1) as c0p, \
             tc.tile_pool(name="c0ps", bufs=2, space="PSUM") as c0ps, \
             tc.tile_pool(name="c0tmp", bufs=2) as c0tmp:
            oT_sb = [c0p.tile([128, L + 2], BF16, name=f"oT{c}") for c in range(10)]
            for c in range(10):
                nc.sync.dma_start(out=oT_sb[c], in_=oT[c * 128:(c + 1) * 128, :])
            w0_sb = [c0p.tile([128, w0T.shape[1]], BF16, name=f"w0_{kc}")
                     for kc in range(30)]
            for k in range(3):
                for c in range(10):
                    nc.sync.dma_start(
                        out=w0_sb[k * 10 + c],
                        in_=w0T[k * CIN + c * 128: k * CIN + (c + 1) * 128, :])
            if mode == "gather":
                x0loc = c0p.tile([128, L], BF16)
                x0src = dram.tile([128, L], BF16)
                x0g = dram.tile([H, L], BF16, addr_space="Shared")
            else:
                x0f = [c0p.tile([128, L + 2], BF16, name=f"x0f{c}", bufs=1)
                       for c in range(8)]
                for c in range(8):
                    nc.vector.memset(x0f[c][:, 0:1], 0.0)
                    nc.vector.memset(x0f[c][:, L + 1:L + 2], 0.0)
            for m in range(n_m0):
                for tb in range(4):
                    ps = c0ps.tile([128, 512], F32, name="c0psum", tag="c0psum")
                    idx = 0
                    for k in range(3):
                        for c in range(10):
                            nc.tensor.matmul(
                                ps,
                                lhsT=w0_sb[k * 10 + c][:, m * 128:(m + 1) * 128]
                                if mode != "gather" else w0_sb[k * 10 + c][:, :],
                                rhs=oT_sb[c][:, tb * 512 + k: tb * 512 + k + 512],
                                start=(idx == 0), stop=(idx == 29))
                            idx += 1
                    if mode == "gather":
                        dst = x0loc[:, tb * 512:(tb + 1) * 512]
                        bias = q0b_sb[:, 0:1]
                    else:
                        dst = x0f[m][:, 1 + tb * 512: 1 + (tb + 1) * 512]
                        bias = q0b_sb[:, m:m + 1]
                    _selu_from_psum(nc, c0tmp, zeros, ps, bias, dst,
                                    128, 512, f"c0_{m}_{tb}")
            if mode == "gather":
                nc.sync.dma_start(out=x0src[:, :], in_=x0loc[:, :])
                nc.gpsimd.collective_compute(
                    "AllGather", ALU.bypass,
                    replica_groups=[list(range(NCORES))],
                    ins=[x0src.opt()], outs=[x0g.opt()])

        # ---------------- conv1 ----------------
        with tc.tile_pool(name="c1", bufs=1) as c1p, \
             tc.tile_pool(name="c1ps", bufs=2, space="PSUM") as c1ps, \
             tc.tile_pool(name="c1tmp", bufs=2) as c1tmp:
            if mode == "gather":
                x0f = [c1p.tile([128, L + 2], BF16, name=f"x0g{c}") for c in range(8)]
                for c in range(8):
                    nc.vector.memset(x0f[c][:, 0:1], 0.0)
                    nc.vector.memset(x0f[c][:, L + 1:L + 2], 0.0)
                    nc.sync.dma_start(out=x0f[c][:, 1:L + 1],
                                      in_=x0g[c * 128:(c + 1) * 128, :])
            w1_sb = [c1p.tile([128, 128], BF16, name=f"w1_{kc}") for kc in range(24)]
            for k in range(3):
                for c in range(8):
                    nc.sync.dma_start(
                        out=w1_sb[k * 8 + c],
                        in_=w1T[k * H + c * 128: k * H + (c + 1) * 128, :])
            for tb in range(4):
                ps = c1ps.tile([128, 512], F32, name="c1psum", tag="c1psum")
                idx = 0
                for k in range(3):
                    for c in range(8):
                        nc.tensor.matmul(
                            ps, lhsT=w1_sb[k * 8 + c][:, :],
                            rhs=x0f[c][:, tb * 512 + k: tb * 512 + k + 512],
                            start=(idx == 0), stop=(idx == 23))
                        idx += 1
                for hh in range(2):
                    _selu_from_psum(
                        nc, c1tmp, zeros,
                        ps[hh * 64:(hh + 1) * 64, :],
                        q1b_sb[hh * 64:(hh + 1) * 64, 0:1],
                        qh[hh][:, tb * 512:(tb + 1) * 512],
                        64, 512, f"c1_{tb}_{hh}", pbase=hh * 64)

        # ------------- attention + scramble -------------
        with tc.tile_pool(name="wo", bufs=1) as wop:
            wo_sb = [wop.tile([128, H], BF16, name=f"wo{k}") for k in range(16)]
            for k in range(16):
                nc.sync.dma_start(out=wo_sb[k],
                                  in_=woT[k * 128:(k + 1) * 128, :])

            with tc.tile_pool(name="attn", bufs=1) as atp, \
                 tc.tile_pool(name="ppool", bufs=20) as ppool, \
                 tc.tile_pool(name="scps", bufs=3, space="PSUM") as scps, \
                 tc.tile_pool(name="mixps", bufs=2, space="PSUM") as mixps, \
                 tc.tile_pool(name="bcps", bufs=2, space="PSUM") as bcps:
                for hh in range(2):
                    # strided rewrite of q into cat layout (pure copies)
                    qre = qh[hh].rearrange("p (pg j) -> p pg j", j=16)
                    for kk in range(8):
                        for jj in range(2):
                            nc.vector.tensor_copy(
                                out=catq[hh][kk][jj * 64:(jj + 1) * 64, :],
                                in_=qre[:, :, kk * 2 + jj])
                    for tb in range(4):
                        plist = []
                        for st in range(16):
                            ps_sc = scps.tile([128, 512], F32, name="ps_sc",
                                              tag="ps_sc")
                            nc.tensor.matmul(
                                ps_sc,
                                lhsT=kvT_sb[hh][:, st * 128:(st + 1) * 128],
                                rhs=qh[hh][:, tb * 512:(tb + 1) * 512],
                                start=True, stop=True)
                            p_t = ppool.tile([128, 512], BF16, name="p_t", tag="p")
                            nc.scalar.activation(p_t, ps_sc, ACTF.Exp, scale=0.125)
                            plist.append(p_t)
                        ps_mix = mixps.tile([65, 512], F32, name="ps_mix",
                                            tag="ps_mix")
                        for st in range(16):
                            nc.tensor.matmul(
                                ps_mix,
                                lhsT=kvag_sb[st][:, hh * 65:(hh + 1) * 65],
                                rhs=plist[st][:, :],
                                start=(st == 0), stop=(st == 15))
                        recip = atp.tile([1, 512], F32, name="recip", tag="recip",
                                         bufs=2)
                        nc.vector.reciprocal(recip, ps_mix[64:65, :])
                        ps_bc = bcps.tile([64, 512], F32, name="ps_bc", tag="ps_bc")
                        nc.tensor.matmul(ps_bc, lhsT=ones, rhs=recip,
                                         start=True, stop=True)
                        bc = atp.tile([64, 512], F32, name="bc", tag="bc", bufs=2)
                        nc.scalar.copy(bc, ps_bc)
                        mre = ps_mix[0:64, :].rearrange("p (pg j) -> p pg j", j=16)
                        bre = bc.rearrange("p (pg j) -> p pg j", j=16)
                        for kk in range(8):
                            for jj in range(2):
                                nc.vector.tensor_tensor(
                                    out=catm[hh][kk][jj * 64:(jj + 1) * 64,
                                                     tb * 32:(tb + 1) * 32],
                                    in0=mre[:, :, kk * 2 + jj],
                                    in1=bre[:, :, kk * 2 + jj],
                                    op=ALU.mult)

            # ------------- out-projection -------------
            with tc.tile_pool(name="ops", bufs=2, space="PSUM") as ops, \
                 tc.tile_pool(name="otmp", bufs=2) as otmp:
                for hh in range(2):
                    for m in range(8):
                        ps_o = ops.tile([128, 128], F32, name="ps_o", tag="ps_o")
                        for k in range(8):
                            nc.tensor.matmul(
                                ps_o, lhsT=wo_sb[k][:, m * 128:(m + 1) * 128],
                                rhs=catm[hh][k][:, :],
                                start=(k == 0), stop=False)
                        for k in range(8):
                            nc.tensor.matmul(
                                ps_o, lhsT=wo_sb[8 + k][:, m * 128:(m + 1) * 128],
                                rhs=catq[hh][k][:, :],
                                start=False, stop=(k == 7))
                        _selu_from_psum(nc, otmp, zeros, ps_o,
                                        outb_sb[:, m:m + 1], aoT[hh][m][:, :],
                                        128, 128, f"o_{hh}_{m}")

        # ------------- V/C logits -------------
        with tc.tile_pool(name="vstream", bufs=24) as vsp, \
             tc.tile_pool(name="vstage", bufs=4) as vst, \
             tc.tile_pool(name="vps", bufs=4, space="PSUM") as vps:
            for vb in range(NVB):
                vtiles = []
                for k in range(8):
                    vt = vsp.tile([128, VBW], BF16, name="vt", tag="vct")
                    nc.sync.dma_start(
                        out=vt,
                        in_=vct[k * 128:(k + 1) * 128, vb * VBW:(vb + 1) * VBW])
                    vtiles.append(vt)
                for hh in range(2):
                    ps_v = vps.tile([128, VBW], F32, name="ps_v", tag="ps_v")
                    for k in range(8):
                        nc.tensor.matmul(ps_v, lhsT=aoT[hh][k][:, :],
                                         rhs=vtiles[k][:, :],
                                         start=(k == 0), stop=(k == 7))
                    st = vst.tile([128, VBW], BF16, name="vstage", tag="vstage")
                    nc.scalar.copy(st, ps_v)
                    nc.sync.dma_start(
                        out=out[hh * 128:(hh + 1) * 128, vb * VBW:(vb + 1) * VBW],
                        in_=st)


# ---------------- host side ----------------

def _wn_conv(v, g):
    n = np.sqrt((v * v).sum(axis=(1, 2), keepdims=True))
    return g[:, None, None] * v / n


def _wn_lin(v, g):
    return g[:, None] * v / np.linalg.norm(v, axis=1, keepdims=True)


def _selu_np(x):
    return np.where(x > 0, LAM * x,
                    LAM * ALPHA * (np.exp(np.minimum(x, 0)) - 1)).astype(np.float32)


def _bf16(x):
    return np.ascontiguousarray(x.astype(ml_dtypes.bfloat16))


def _f32(x):
    return np.ascontiguousarray(x.astype(np.float32))


_PROGRAM_CACHE = {}


def kernel(o, f, q0_v, q0_g, q0_b, q1_v, q1_g, q1_b,
           out_v, out_g, out_b, V_v, V_g, V_b, C_v, C_g, C_b):
    o, f = np.asarray(o), np.asarray(f)

    w0 = _wn_conv(np.asarray(q0_v), np.asarray(q0_g)) * LAM      # (H, CIN, 3)
    w1 = _wn_conv(np.asarray(q1_v), np.asarray(q1_g)) * LAM      # (H, H, 3)
    b0 = np.asarray(q0_b) * LAM
    b1 = np.asarray(q1_b) * LAM
    woutT = np.ascontiguousarray(_wn_lin(np.asarray(out_v), np.asarray(out_g)).T) * LAM
    outb_l = np.asarray(out_b) * LAM
    vc = np.concatenate([_wn_lin(np.asarray(V_v), np.asarray(V_g)),
                         _wn_lin(np.asarray(C_v), np.asarray(C_g))], axis=0)
    vct = np.ascontiguousarray(vc.T)                             # (H, 32512)
    kv = _selu_np(f)                                             # (S, H)

    oT_pad = np.zeros((CIN, L + 2), np.float32)
    oT_pad[:, 1:L + 1] = o.T
    oT_b = _bf16(oT_pad)
    w0T = _bf16(w0.transpose(2, 1, 0).reshape(3 * CIN, H))       # (3840, 1024)
    w1T = _bf16(w1.transpose(2, 1, 0).reshape(3 * H, H))         # (3072, 1024)
    woutT_b = _bf16(woutT)
    vct_b = _bf16(vct)
    kvT_full = np.ascontiguousarray(kv.T)                        # (H, S)
    outb_dev = _f32(outb_l.reshape(8, 128).T)                    # (128, 8)

    mode = MODE
    key = mode
    if key not in _PROGRAM_CACHE:
        _PROGRAM_CACHE[key] = build_program(mode)
    nc = _PROGRAM_CACHE[key]

    in_maps = []
    for i in range(NCORES):
        sl = slice(i * 128, (i + 1) * 128)
        kvag = np.zeros((S, 130), np.float32)
        for hh in range(2):
            n = 2 * i + hh
            kvag[:, hh * 65:hh * 65 + 64] = kv[:, n * 64:(n + 1) * 64]
            kvag[:, hh * 65 + 64] = 1.0
        m = {
            "oT": oT_b,
            "w1T": _bf16(w1.transpose(2, 1, 0).reshape(3 * H, H)[:, sl]),
            "q1b": _f32(b1[sl][:, None]),
            "kvT": _bf16(kvT_full[sl, :]),
            "kvag": _bf16(kvag),
            "woT": woutT_b,
            "outb": outb_dev,
            "vct": vct_b,
        }
        if mode == "gather":
            m["w0T"] = _bf16(w0.transpose(2, 1, 0).reshape(3 * CIN, H)[:, sl])
            m["q0b"] = _f32(b0[sl][:, None])
        else:
            m["w0T"] = w0T
            m["q0b"] = _f32(b0.reshape(8, 128).T)
        in_maps.append(m)

    kwargs = {}
    if os.environ.get("NN_COPY_TRACE", "0") == "1":
        kwargs = dict(trace=True)
    res = run_bass_kernel_spmd(nc, in_maps, core_ids=list(range(NCORES)), **kwargs)
    global LAST_RESULTS
    LAST_RESULTS = res
    shards = [np.asarray(res.results[i]["out"]).astype(np.float32)
              for i in range(NCORES)]
    full = np.concatenate(shards, axis=0)                        # (2048, 32512)
    full += np.concatenate([np.asarray(V_b), np.asarray(C_b)])[None, :]
    return full


# revision 17
# speedup vs baseline: 1.3745x; 1.3745x over previous
"""Trainium2 Bass kernel for nn_Copy_56470230008202 (sparse_attention).

Strategy (8 NeuronCores, SPMD, one launch, pipelined AllGather):
  The reference's `mixh.reshape(1,-1,H)` / `q2 = qh.transpose(1,0,2,3).reshape(-1,1,H)`
  views scramble rows so that output row l' = n*128 + pg (head n, position
  group pg) draws features from positions t = pg*16 + j of head n only.
  Hence: core i owns heads {2i, 2i+1} == output rows [i*256, (i+1)*256).

  - conv0 (CIN->H, k=3): channel-sharded; core i computes x0 channels
    [128i, 128i+128) over all L from the (replicated) input o.
  - AllGather x0 per 512-column chunk (pipelined with conv0).
  - conv1 (H->H, k=3): core i computes only its 128 q-channels (2 heads).
  - attention per head (scoresT layout [s', t]; softmax over partitions via
    an appended ones-column in kv for the denominator; no max subtraction --
    scores are in [-6, 6]); mix accumulated over s' tiles.
  - scramble rewrite into cat layout via strided DVE writes.
  - out-proj + SELU -> aoT [c_out, l'], then V/C logits vs full VC^T,
    streamed in contiguous 520 KB blocks; psum evacuated as bf16 and
    written to a bf16 output, cast to fp32 + bias on host.
  All matmuls bf16 inputs / fp32 PSUM accumulation. Weight-norm, selu(f),
  transposes, per-tile contiguous repacking, sharding and the final bias
  add run on host. All DRAM tensors are packed so every DMA is a single
  large contiguous block (descriptor-efficient).
"""

import os
import sys

for _p in ("/opt/trn_rl_repo", "/root/.axon_site/_ro/trn_rl_repo"):
    if os.path.isdir(_p) and _p not in sys.path:
        sys.path.append(_p)

import numpy as np
import ml_dtypes

import concourse.bass as bass
import concourse.mybir as mybir
from concourse import bacc
from concourse.tile import TileContext
from concourse.bass_utils import run_bass_kernel_spmd

F32 = mybir.dt.float32
BF16 = mybir.dt.bfloat16
ALU = mybir.AluOpType
ACTF = mybir.ActivationFunctionType

H, NH, HD = 1024, 16, 64
CIN, VOCAB, LIMIT, L, S = 1280, 32000, 512, 2048, 2048
VC = VOCAB + LIMIT              # 32512 = 16 groups * 2032 = 64 * 508
NVB, VBW = 64, 508
NG, GW = 16, 2032               # V-stream groups: 4 vocab blocks per group
NCORES = 8
LAM, ALPHA = 1.0507009873554805, 1.6732632423543772


def _selu_from_psum(nc, tmp, psum_ap, bias_ap, out_ap, P, N, idx, pbase=0,
                    zeros=None):
    """out = selu(z) given psum = LAM*z (lambda folded into weights+bias).
    selu(z) = max(y,0) + LAM*ALPHA*(exp(min(y,0)/LAM) - 1),  y = LAM*z + b'.
    pbase: base partition of bias_ap -- SBUF operands of one instruction
    must share their base partition (walrus NCC_IBIR297).
    """
    m = tmp.tile([P, N], F32, name=f"selu_m{idx}", tag=f"selu_m{P}x{N}")
    r = tmp.tile([P, N], F32, name=f"selu_r{idx}", tag=f"selu_r{P}x{N}")
    e = tmp.tile([P, N], F32, name=f"selu_e{idx}", tag=f"selu_e{P}x{N}")
    t = tmp.tile([P, N], F32, name=f"selu_t{idx}", tag=f"selu_t{P}x{N}")
    z = zeros[pbase:pbase + P, :N]
    nc.vector.scalar_tensor_tensor(m, psum_ap, bias_ap, z, op0=ALU.add, op1=ALU.min)
    nc.vector.scalar_tensor_tensor(r, psum_ap, bias_ap, z, op0=ALU.add, op1=ALU.max)
    nc.scalar.activation(e, m, ACTF.Exp, scale=1.0 / LAM)
    nc.vector.tensor_scalar(t, e, LAM * ALPHA, -LAM * ALPHA, op0=ALU.mult, op1=ALU.add)
    nc.vector.tensor_tensor(out_ap, t, r, op=ALU.add)


def build_program():
    nc = bacc.Bacc("TRN2", target_bir_lowering=False, debug=False,
                   num_devices=NCORES)
    # all inputs packed per-SBUF-tile contiguous (column blocks)
    oTp = nc.declare_dram_parameter("oTp", [128, 10 * (L + 2)], BF16, isOutput=False)
    w0p = nc.declare_dram_parameter("w0p", [128, 3840], BF16, isOutput=False)
    w1p = nc.declare_dram_parameter("w1p", [128, 3072], BF16, isOutput=False)
    kvp = nc.declare_dram_parameter("kvp", [64, 2 * S], BF16, isOutput=False)
    kvagp = nc.declare_dram_parameter("kvagp", [128, 16 * 130], BF16, isOutput=False)
    wop = nc.declare_dram_parameter("wop", [128, 16 * 1024], BF16, isOutput=False)
    cst = nc.declare_dram_parameter("cst", [128, 10], F32, isOutput=False)
    vctp = nc.declare_dram_parameter("vctp", [NG, 8, 128, GW], BF16, isOutput=False)
    out = nc.declare_dram_parameter("out", [2, NG, 128, GW], BF16, isOutput=True)

    with TileContext(nc) as tc:
        _emit(tc, oTp, w0p, w1p, kvp, kvagp, wop, cst, vctp, out)
    if not nc.is_finalized():
        nc.finalize()
    return nc


def _emit(tc, oTp, w0p, w1p, kvp, kvagp, wop, cst, vctp, out):
    nc = tc.nc

    with tc.tile_pool(name="const", bufs=1) as constp, \
         tc.tile_pool(name="persist", bufs=1) as pers, \
         tc.tile_pool(name="dram", bufs=1, space="DRAM") as dram:
        zeros = constp.tile([128, 512], F32)
        nc.vector.memset(zeros, 0.0)
        cst_sb = constp.tile([128, 10], F32)
        nc.sync.dma_start(out=cst_sb, in_=cst[:, :])
        q0b_sb = cst_sb[:, 0:1]
        q1b_sb = cst_sb[:, 1:2]
        outb_sb = cst_sb[:, 2:10]

        # persistent activations
        qh = [pers.tile([64, L], BF16, name=f"qh{hh}") for hh in range(2)]
        catm = [[pers.tile([128, 128], BF16, name=f"catm{hh}_{kk}")
                 for kk in range(8)] for hh in range(2)]
        catq = [[pers.tile([128, 128], BF16, name=f"catq{hh}_{kk}")
                 for kk in range(8)] for hh in range(2)]
        aoT = [[pers.tile([128, 128], BF16, name=f"aoT{hh}_{kk}")
                for kk in range(8)] for hh in range(2)]
        kvT_sb = pers.tile([64, 2 * S], BF16)
        kvag_sb = pers.tile([128, 16 * 130], BF16)

        # ---------------- conv0 + AllGather ----------------
        with tc.tile_pool(name="c0", bufs=1) as c0p, \
             tc.tile_pool(name="c0ps", bufs=3, space="PSUM") as c0ps, \
             tc.tile_pool(name="c0tmp", bufs=2) as c0tmp:
            # chunked loads so the first matmuls start after ~1 MB, not 7 MB
            oT_sb = c0p.tile([128, 10 * (L + 2)], BF16)
            w0_sb = c0p.tile([128, 3840], BF16)
            nc.sync.dma_start(out=w0_sb[:, 0:1280], in_=w0p[:, 0:1280])
            nc.sync.dma_start(out=oT_sb[:, 0:L + 2], in_=oTp[:, 0:L + 2])
            for k in range(1, 3):
                nc.sync.dma_start(out=w0_sb[:, k * 1280:(k + 1) * 1280],
                                  in_=w0p[:, k * 1280:(k + 1) * 1280])
            for c in range(1, 10):
                nc.sync.dma_start(
                    out=oT_sb[:, c * (L + 2):(c + 1) * (L + 2)],
                    in_=oTp[:, c * (L + 2):(c + 1) * (L + 2)])
            # loads not needed until attention go last
            nc.sync.dma_start(out=kvT_sb, in_=kvp[:, :])
            nc.sync.dma_start(out=kvag_sb, in_=kvagp[:, :])
            x0loc = c0p.tile([128, L], BF16)
            PIPE_GATHER = os.environ.get("NN_COPY_PIPE_GATHER", "0") == "1"
            if PIPE_GATHER:
                x0src = [dram.tile([128, 512], BF16, name=f"x0src{tb}")
                         for tb in range(4)]
                x0g = [dram.tile([H, 512], BF16, name=f"x0g{tb}",
                                 addr_space="Shared") for tb in range(4)]
            else:
                x0src1 = dram.tile([128, L], BF16, name="x0src1")
                x0g1 = dram.tile([H, L], BF16, name="x0g1", addr_space="Shared")
            for tb in range(4):
                ps = c0ps.tile([128, 512], F32, name="c0psum", tag="c0psum")
                idx = 0
                for c in range(10):
                    for k in range(3):
                        nc.tensor.matmul(
                            ps, lhsT=w0_sb[:, (k * 10 + c) * 128:
                                           (k * 10 + c + 1) * 128],
                            rhs=oT_sb[:, c * (L + 2) + tb * 512 + k:
                                      c * (L + 2) + tb * 512 + k + 512],
                            start=(idx == 0), stop=(idx == 29))
                        idx += 1
                dst = x0loc[:, tb * 512:(tb + 1) * 512]
                _selu_from_psum(nc, c0tmp, ps, q0b_sb, dst, 128, 512,
                                f"c0_{tb}", zeros=zeros)
                if PIPE_GATHER:
                    nc.sync.dma_start(out=x0src[tb][:, :], in_=dst)
                    nc.gpsimd.collective_compute(
                        "AllGather", ALU.bypass,
                        replica_groups=[list(range(NCORES))],
                        ins=[x0src[tb].opt()], outs=[x0g[tb].opt()])
            if not PIPE_GATHER:
                nc.sync.dma_start(out=x0src1[:, :], in_=x0loc[:, :])
                nc.gpsimd.collective_compute(
                    "AllGather", ALU.bypass,
                    replica_groups=[list(range(NCORES))],
                    ins=[x0src1.opt()], outs=[x0g1.opt()])

        # ---------------- conv1 ----------------
        with tc.tile_pool(name="c1", bufs=1) as c1p, \
             tc.tile_pool(name="c1ps", bufs=3, space="PSUM") as c1ps, \
             tc.tile_pool(name="c1tmp", bufs=2) as c1tmp:
            x0f = [c1p.tile([128, L + 2], BF16, name=f"x0g{c}") for c in range(8)]
            for c in range(8):
                nc.vector.memset(x0f[c][:, 0:1], 0.0)
                nc.vector.memset(x0f[c][:, L + 1:L + 2], 0.0)
                if PIPE_GATHER:
                    for tb in range(4):
                        nc.sync.dma_start(
                            out=x0f[c][:, 1 + tb * 512: 1 + (tb + 1) * 512],
                            in_=x0g[tb][c * 128:(c + 1) * 128, :])
                else:
                    nc.sync.dma_start(
                        out=x0f[c][:, 1:L + 1],
                        in_=x0g1[c * 128:(c + 1) * 128, :])
            w1_sb = c1p.tile([128, 3072], BF16)
            nc.sync.dma_start(out=w1_sb, in_=w1p[:, :])
            for tb in range(4):
                ps = c1ps.tile([128, 512], F32, name="c1psum", tag="c1psum")
                idx = 0
                for k in range(3):
                    for c in range(8):
                        nc.tensor.matmul(
                            ps, lhsT=w1_sb[:, (k * 8 + c) * 128:
                                           (k * 8 + c + 1) * 128],
                            rhs=x0f[c][:, tb * 512 + k: tb * 512 + k + 512],
                            start=(idx == 0), stop=(idx == 23))
                        idx += 1
                for hh in range(2):
                    _selu_from_psum(
                        nc, c1tmp,
                        ps[hh * 64:(hh + 1) * 64, :],
                        q1b_sb[hh * 64:(hh + 1) * 64, :],
                        qh[hh][:, tb * 512:(tb + 1) * 512],
                        64, 512, f"c1_{tb}_{hh}", pbase=hh * 64, zeros=zeros)

        # ------------- attention + scramble -------------
        with tc.tile_pool(name="wo", bufs=1) as wop_:
            wo_sb = wop_.tile([128, 16 * 1024], BF16)
            nc.sync.dma_start(out=wo_sb, in_=wop[:, :])

            with tc.tile_pool(name="attn", bufs=1) as atp, \
                 tc.tile_pool(name="ppool", bufs=24) as ppool, \
                 tc.tile_pool(name="scps", bufs=4, space="PSUM") as scps, \
                 tc.tile_pool(name="mixps", bufs=3, space="PSUM") as mixps:
                for hh in range(2):
                    qre = qh[hh].rearrange("p (pg j) -> p pg j", j=16)
                    for kk in range(8):
                        for jj in range(2):
                            nc.vector.tensor_copy(
                                out=catq[hh][kk][jj * 64:(jj + 1) * 64, :],
                                in_=qre[:, :, kk * 2 + jj])
                    for tb in range(4):
                        plist = []
                        for st in range(16):
                            ps_sc = scps.tile([128, 512], F32, name="ps_sc",
                                              tag="ps_sc")
                            nc.tensor.matmul(
                                ps_sc,
                                lhsT=kvT_sb[:, hh * S + st * 128:
                                            hh * S + (st + 1) * 128],
                                rhs=qh[hh][:, tb * 512:(tb + 1) * 512],
                                start=True, stop=True)
                            p_t = ppool.tile([128, 512], BF16, name="p_t", tag="p")
                            nc.scalar.activation(p_t, ps_sc, ACTF.Exp, scale=0.125)
                            plist.append(p_t)
                        ps_mix = mixps.tile([65, 512], F32, name="ps_mix",
                                            tag="ps_mix")
                        for st in range(16):
                            nc.tensor.matmul(
                                ps_mix,
                                lhsT=kvag_sb[:, st * 130 + hh * 65:
                                             st * 130 + (hh + 1) * 65],
                                rhs=plist[st][:, :],
                                start=(st == 0), stop=(st == 15))
                        recip = atp.tile([1, 512], F32, name="recip", tag="recip",
                                         bufs=3)
                        nc.vector.reciprocal(recip, ps_mix[64:65, :])
                        bc = atp.tile([64, 512], F32, name="bc", tag="bc", bufs=3)
                        nc.gpsimd.partition_broadcast(bc, recip)
                        mre = ps_mix[0:64, :].rearrange("p (pg j) -> p pg j", j=16)
                        bre = bc.rearrange("p (pg j) -> p pg j", j=16)
                        for kk in range(8):
                            for jj in range(2):
                                nc.vector.tensor_tensor(
                                    out=catm[hh][kk][jj * 64:(jj + 1) * 64,
                                                     tb * 32:(tb + 1) * 32],
                                    in0=mre[:, :, kk * 2 + jj],
                                    in1=bre[:, :, kk * 2 + jj],
                                    op=ALU.mult)

            # ------------- out-projection -------------
            with tc.tile_pool(name="ops", bufs=2, space="PSUM") as ops, \
                 tc.tile_pool(name="otmp", bufs=2) as otmp:
                for hh in range(2):
                    for m in range(8):
                        ps_o = ops.tile([128, 128], F32, name="ps_o", tag="ps_o")
                        for k in range(8):
                            nc.tensor.matmul(
                                ps_o,
                                lhsT=wo_sb[:, k * 1024 + m * 128:
                                           k * 1024 + (m + 1) * 128],
                                rhs=catm[hh][k][:, :],
                                start=(k == 0), stop=False)
                        for k in range(8):
                            nc.tensor.matmul(
                                ps_o,
                                lhsT=wo_sb[:, (8 + k) * 1024 + m * 128:
                                           (8 + k) * 1024 + (m + 1) * 128],
                                rhs=catq[hh][k][:, :],
                                start=False, stop=(k == 7))
                        _selu_from_psum(nc, otmp, ps_o, outb_sb[:, m:m + 1],
                                        aoT[hh][m][:, :], 128, 128,
                                        f"o_{hh}_{m}", zeros=zeros)

        # ------------- V/C logits (streamed, 4 vocab blocks per group) ----
        with tc.tile_pool(name="vstream", bufs=24) as vsp, \
             tc.tile_pool(name="vstage", bufs=6) as vst, \
             tc.tile_pool(name="vps", bufs=6, space="PSUM") as vps:
            for g in range(NG):
                vtiles = []
                for k in range(8):
                    vt = vsp.tile([128, GW], BF16, name="vt", tag="vct")
                    nc.sync.dma_start(out=vt, in_=vctp[g, k, :, :])
                    vtiles.append(vt)
                for hh in range(2):
                    stg = vst.tile([128, GW], BF16, name="vstage", tag="vstage")
                    for u in range(4):
                        ps_v = vps.tile([128, VBW], F32, name="ps_v", tag="ps_v")
                        for k in range(8):
                            nc.tensor.matmul(
                                ps_v, lhsT=aoT[hh][k][:, :],
                                rhs=vtiles[k][:, u * VBW:(u + 1) * VBW],
                                start=(k == 0), stop=(k == 7))
                        dstg = stg[:, u * VBW:(u + 1) * VBW]
                        if hh == 0:
                            nc.scalar.copy(dstg, ps_v)
                        else:
                            nc.vector.tensor_copy(out=dstg, in_=ps_v)
                    nc.sync.dma_start(out=out[hh, g, :, :], in_=stg)


# ---------------- host side ----------------

def _wn_conv(v, g):
    n = np.sqrt((v * v).sum(axis=(1, 2), keepdims=True))
    return g[:, None, None] * v / n


def _wn_lin(v, g):
    return g[:, None] * v / np.linalg.norm(v, axis=1, keepdims=True)


def _selu_np(x):
    return np.where(x > 0, LAM * x,
                    LAM * ALPHA * (np.exp(np.minimum(x, 0)) - 1)).astype(np.float32)


def _bf16(x):
    return np.ascontiguousarray(x.astype(ml_dtypes.bfloat16))


def _f32(x):
    return np.ascontiguousarray(x.astype(np.float32))


_PROGRAM_CACHE = {}


def kernel(o, f, q0_v, q0_g, q0_b, q1_v, q1_g, q1_b,
           out_v, out_g, out_b, V_v, V_g, V_b, C_v, C_g, C_b):
    o, f = np.asarray(o), np.asarray(f)

    w0 = _wn_conv(np.asarray(q0_v), np.asarray(q0_g)) * LAM      # (H, CIN, 3)
    w1 = _wn_conv(np.asarray(q1_v), np.asarray(q1_g)) * LAM      # (H, H, 3)
    b0 = np.asarray(q0_b) * LAM
    b1 = np.asarray(q1_b) * LAM
    woutT = np.ascontiguousarray(_wn_lin(np.asarray(out_v), np.asarray(out_g)).T) * LAM
    outb_l = np.asarray(out_b) * LAM
    vc = np.concatenate([_wn_lin(np.asarray(V_v), np.asarray(V_g)),
                         _wn_lin(np.asarray(C_v), np.asarray(C_g))], axis=0)
    vct = np.ascontiguousarray(vc.T)                             # (H, 32512)
    kv = _selu_np(f)                                             # (S, H)

    # packed layouts (every SBUF tile contiguous in DRAM)
    oT_pad = np.zeros((CIN, L + 2), np.float32)
    oT_pad[:, 1:L + 1] = o.T
    oTp = _bf16(oT_pad.reshape(10, 128, L + 2).transpose(1, 0, 2)
                .reshape(128, 10 * (L + 2)))
    w0T = w0.transpose(2, 1, 0).reshape(3 * CIN, H)              # (3840, 1024)
    w1T = w1.transpose(2, 1, 0).reshape(3 * H, H)                # (3072, 1024)
    wopk = _bf16(woutT.reshape(16, 128, 1024).transpose(1, 0, 2)
                 .reshape(128, 16 * 1024))
    vctp = _bf16(vct.reshape(8, 128, NG, GW).transpose(2, 0, 1, 3))
    kvT_full = np.ascontiguousarray(kv.T)                        # (H, S)

    if "nc" not in _PROGRAM_CACHE:
        _PROGRAM_CACHE["nc"] = build_program()
    nc = _PROGRAM_CACHE["nc"]

    in_maps = []
    for i in range(NCORES):
        sl = slice(i * 128, (i + 1) * 128)
        kvag = np.zeros((S, 130), np.float32)
        for hh in range(2):
            n = 2 * i + hh
            kvag[:, hh * 65:hh * 65 + 64] = kv[:, n * 64:(n + 1) * 64]
            kvag[:, hh * 65 + 64] = 1.0
        kvagp = _bf16(kvag.reshape(16, 128, 130).transpose(1, 0, 2)
                      .reshape(128, 16 * 130))
        w0pi = _bf16(w0T[:, sl].reshape(30, 128, 128).transpose(1, 0, 2)
                     .reshape(128, 3840))
        w1pi = _bf16(w1T[:, sl].reshape(24, 128, 128).transpose(1, 0, 2)
                     .reshape(128, 3072))
        kvpi = _bf16(kvT_full[sl, :].reshape(2, 64, S).transpose(1, 0, 2)
                     .reshape(64, 2 * S))
        csti = np.zeros((128, 10), np.float32)
        csti[:, 0] = b0[sl]
        csti[:, 1] = b1[sl]
        csti[:, 2:10] = outb_l.reshape(8, 128).T
        in_maps.append({
            "oTp": oTp,
            "w0p": w0pi,
            "w1p": w1pi,
            "kvp": kvpi,
            "kvagp": kvagp,
            "wop": wopk,
            "cst": _f32(csti),
            "vctp": vctp,
        })

    kwargs = {}
    if os.environ.get("NN_COPY_TRACE", "0") == "1":
        kwargs = dict(trace=True)
    res = run_bass_kernel_spmd(nc, in_maps, core_ids=list(range(NCORES)), **kwargs)
    global LAST_RESULTS
    LAST_RESULTS = res
    shards = []
    for i in range(NCORES):
        od = np.asarray(res.results[i]["out"]).astype(np.float32)  # (2,NG,128,GW)
        shards.append(od.transpose(0, 2, 1, 3).reshape(256, VC))
    full = np.concatenate(shards, axis=0)                        # (2048, 32512)
    full += np.concatenate([np.asarray(V_b), np.asarray(C_b)])[None, :]
    return full
